# revision 4
# baseline (speedup 1.0000x reference)
"""Trainium2 Bass kernel for nn_MetaNet (triu-gram features -> Wh matvec ->
relu -> 14 per-head linears), distributed over 8 NeuronCores.

v10 = v9 + setup/hraw DMAs on scalar ring + split output drain.

v3 — fixes the two stage-2 serial bottlenecks found in the v2 trace
(126 single-lane DVE psum drains @795ns + 126 small out-DMAs each costing
~700ns of Sync-engine issue time) and the DMA stall during the AllGather:

  * Stage 1 (unchanged math): feat grid [128,595] bf16 in SBUF, Wh streams
    as 17 DMAs of [128, 35*224] bf16 (~1.96MB each), 595 accumulating
    matmuls (moving operand = Wh slice, N=224) into psum [1,224].
  * Collective-dependent DMAs (cc_in store, hidden reload) are issued from
    GpSimd (SWDGE) so the Sync-engine HWDGE FIFO (wh then wf streams)
    never blocks on the gather -> Wf prefetch fills the ~35us gather
    bubble (wfp bufs=21 holds the whole per-core Wf shard, 16.5MB).
  * Stage 2 transposed matmuls: out_chunk[128,1] = wf[128,128chunk].T @
    hsel[:,j] with N=1, M=128. All 504 output columns land in ONE psum
    bank [128,504]; a single DVE copy + a single 258KB DMA drain the
    whole stage-2 output (engages all 128 DVE lanes instead of 1).
"""

import math
from dataclasses import dataclass

import numpy as np
import ml_dtypes

BF16 = ml_dtypes.bfloat16


@dataclass(frozen=True)
class Cfg:
    n_cores: int = 8
    cs: tuple = (64, 128, 256, 256)
    hid: int = 128
    nl: int = 14
    d2: int = 36864
    tile_n2: int = 512      # output cols per stage-2 tile
    qn: int = 128           # output cols per stage-2 matmul (psum column)
    g1k: int = 35           # stage-1 feat chunks per big DMA group
    nA: int = 16            # big groups (16*35 = 560 chunks)
    gB: int = 7             # chunks per small tail group (5*7 = 35)
    g2k: int = 6            # stage-2 tiles per DMA group
    run: int = 18           # stage-2 tiles per head-run (126 = 7*18)

    @property
    def feat_len(self):
        return sum(c * (c + 1) // 2 for c in self.cs)  # 76128

    @property
    def nk(self):
        return math.ceil(self.feat_len / 128)  # 595

    @property
    def hidden(self):
        return self.hid * self.nl  # 1792

    @property
    def rows(self):
        assert self.hidden % self.n_cores == 0
        return self.hidden // self.n_cores  # 224

    @property
    def nB(self):  # small tail groups
        rem = self.nk - self.nA * self.g1k
        assert rem % self.gB == 0
        return rem // self.gB  # 5

    @property
    def t2(self):
        total = self.nl * self.d2
        assert total % (self.n_cores * self.tile_n2) == 0
        return total // (self.n_cores * self.tile_n2)  # 126

    @property
    def ng2(self):
        assert self.t2 % self.g2k == 0
        return self.t2 // self.g2k  # 21

    @property
    def nruns(self):
        assert self.t2 % self.run == 0
        return self.t2 // self.run  # 7

    @property
    def nq(self):  # stage-2 matmuls per tile
        assert self.tile_n2 % self.qn == 0
        return self.tile_n2 // self.qn  # 4

    @property
    def outc(self):  # total stage-2 psum columns (<= 512, one bank)
        v = self.t2 * self.nq  # 504
        assert v <= 512
        return v

    @property
    def ntile_per_head(self):
        assert self.d2 % self.tile_n2 == 0
        return self.d2 // self.tile_n2  # 72


FULL = Cfg()


def build_nc(cfg: Cfg, debug_taps: bool = False):
    import concourse.bacc as bacc
    import concourse.tile as tile
    import concourse.mybir as mybir

    f32 = mybir.dt.float32
    bf16 = mybir.dt.bfloat16
    nc = bacc.Bacc("TRN2", target_bir_lowering=False, debug=False,
                   num_devices=cfg.n_cores)

    featg = nc.dram_tensor("featg", [128, cfg.nk], bf16, kind="ExternalInput")
    wha = nc.dram_tensor("wha", [cfg.nA, 128, cfg.g1k * cfg.rows], bf16,
                         kind="ExternalInput")
    whb = nc.dram_tensor("whb", [cfg.nB, 128, cfg.gB * cfg.rows], bf16,
                         kind="ExternalInput")
    bht = nc.dram_tensor("bht", [cfg.nl, cfg.hid], f32, kind="ExternalInput")
    sel = nc.dram_tensor("sel", [cfg.nl, cfg.nruns], bf16,
                         kind="ExternalInput")
    wf = nc.dram_tensor("wf", [cfg.ng2, cfg.hid, cfg.g2k * cfg.tile_n2],
                        bf16, kind="ExternalInput")
    out = nc.dram_tensor("out", [128, cfg.outc], f32, kind="ExternalOutput")
    if debug_taps:
        dbg_hpart = nc.dram_tensor("dbg_hpart", [cfg.rows], f32,
                                   kind="ExternalOutput")
        dbg_hsel = nc.dram_tensor("dbg_hsel", [cfg.hid, cfg.nruns], f32,
                                  kind="ExternalOutput")

    with tile.TileContext(nc) as tc:
        with (
            tc.tile_pool(name="const", bufs=1) as const,
            tc.tile_pool(name="whp", bufs=3) as whp,
            tc.tile_pool(name="whsp", bufs=2) as whsp,
            tc.tile_pool(name="wfp", bufs=21) as wfp,
            tc.tile_pool(name="ps1", bufs=1, space="PSUM") as ps1p,
            tc.tile_pool(name="psh", bufs=1, space="PSUM") as pshp,
            tc.tile_pool(name="ps2", bufs=1, space="PSUM") as ps2p,
            tc.tile_pool(name="dram", bufs=1, space="DRAM") as dram,
        ):
            featg_t = const.tile([128, cfg.nk], bf16)
            nc.scalar.dma_start(featg_t[:], featg[:])
            bht_t = const.tile([cfg.nl, cfg.hid], f32)
            nc.scalar.dma_start(bht_t[:], bht[:])
            sel_t = const.tile([cfg.nl, cfg.nruns], bf16)
            nc.scalar.dma_start(sel_t[:], sel[:])

            # ---- stage 1: partial hidden [1, rows] over 595 feat chunks ----
            psum1 = ps1p.tile([1, cfg.rows], f32)
            k = 0
            for g in range(cfg.nA):
                whg = whp.tile([128, cfg.g1k * cfg.rows], bf16, tag="whg")
                nc.sync.dma_start(whg[:], wha[g])
                for t in range(cfg.g1k):
                    nc.tensor.matmul(
                        psum1[:], featg_t[:, k:k + 1],
                        whg[:, t * cfg.rows:(t + 1) * cfg.rows],
                        start=(k == 0), stop=(k == cfg.nk - 1))
                    k += 1
            # small tail groups: the last DMA->matmul drain into the
            # collective trigger is ~0.8us instead of ~4us
            for g in range(cfg.nB):
                whs = whsp.tile([128, cfg.gB * cfg.rows], bf16, tag="whs")
                nc.sync.dma_start(whs[:], whb[g])
                for t in range(cfg.gB):
                    nc.tensor.matmul(
                        psum1[:], featg_t[:, k:k + 1],
                        whs[:, t * cfg.rows:(t + 1) * cfg.rows],
                        start=(k == 0), stop=(k == cfg.nk - 1))
                    k += 1

            hpart = const.tile([1, cfg.rows], f32)
            nc.vector.tensor_copy(hpart[:], psum1[:])
            # gather-dependent DMAs go via GpSimd/SWDGE so the Sync HWDGE
            # FIFO (carrying the wf prefetch stream) never stalls on them.
            cc_in = dram.tile([1, cfg.rows], f32)
            nc.scalar.dma_start(cc_in[:], hpart[:])
            cc_out = dram.tile([cfg.n_cores, cfg.rows], f32,
                               addr_space="Shared")
            nc.gpsimd.collective_compute(
                "AllGather", mybir.AluOpType.bypass,
                replica_groups=[list(range(cfg.n_cores))],
                ins=[cc_in[:].opt()], outs=[cc_out[:].opt()],
            )
            hraw = const.tile([cfg.nl, cfg.hid], f32)
            nc.scalar.dma_start(
                hraw[:],
                cc_out[:].rearrange("a b -> (a b)").rearrange(
                    "(n p) -> n p", p=cfg.hid))
            hsum = const.tile([cfg.nl, cfg.hid], f32)
            nc.vector.tensor_add(hsum[:], hraw[:], bht_t[:])
            hstat = const.tile([cfg.nl, cfg.hid], bf16)
            nc.scalar.activation(hstat[:], hsum[:],
                                 mybir.ActivationFunctionType.Relu)

            psumh = pshp.tile([cfg.hid, cfg.nruns], f32)
            nc.tensor.matmul(psumh[:], hstat[:], sel_t[:],
                             start=True, stop=True)
            hsel = const.tile([cfg.hid, cfg.nruns], bf16)
            nc.vector.tensor_copy(hsel[:], psumh[:])

            if debug_taps:
                nc.sync.dma_start(dbg_hpart[:], hpart[0, :])
                nc.sync.dma_start(dbg_hsel[:], psumh[:])

            # ---- stage 2: 504 transposed matmuls into two psum banks ----
            h1 = cfg.outc // 2
            ps2a = ps2p.tile([128, h1], f32, tag="ps2a")
            ps2b = ps2p.tile([128, cfg.outc - h1], f32, tag="ps2b")
            for j in range(cfg.nruns):
                for t in range(cfg.run):
                    u = j * cfg.run + t
                    g, s = divmod(u, cfg.g2k)
                    if s == 0:
                        wfg = wfp.tile([cfg.hid, cfg.g2k * cfg.tile_n2],
                                       bf16, tag="wfg")
                        nc.sync.dma_start(wfg[:], wf[g])
                    for q in range(cfg.nq):
                        c = u * cfg.nq + q
                        off = s * cfg.tile_n2 + q * cfg.qn
                        pst = ps2a[:, c:c + 1] if c < h1 else \
                            ps2b[:, c - h1:c - h1 + 1]
                        nc.tensor.matmul(
                            pst,
                            wfg[:, off:off + cfg.qn],
                            hsel[:, j:j + 1],
                            start=True, stop=True)
            ot = const.tile([128, cfg.outc], f32)
            nc.vector.tensor_copy(ot[:, :h1], ps2a[:])
            nc.sync.dma_start(out[:, :h1], ot[:, :h1])
            nc.vector.tensor_copy(ot[:, h1:], ps2b[:])
            nc.sync.dma_start(out[:, h1:], ot[:, h1:])

    nc.compile()
    return nc


def _assemble_feat(cfg: Cfg, g1, g2, g3, g4):
    """avgpool(g4) + packed triu features -> bf16 grid [128, nk]."""
    g4 = g4.reshape(2 * cfg.cs[3], 2 * cfg.cs[3]).astype(np.float32)
    g4p = g4.reshape(cfg.cs[3], 2, cfg.cs[3], 2).mean(axis=(1, 3))
    parts = []
    for c, g in zip(cfg.cs, [g1, g2, g3, g4p]):
        m = np.asarray(g).reshape(c, c)
        r, co = np.triu_indices(c)
        parts.append(m[r, co])
    feat = np.concatenate(parts).astype(np.float32)
    assert feat.shape[0] == cfg.feat_len
    grid = np.zeros(128 * cfg.nk, dtype=np.float32)
    grid[:cfg.feat_len] = feat
    return np.ascontiguousarray(grid.reshape(128, cfg.nk)).astype(BF16)


def shard_inputs(cfg: Cfg, g1, g2, g3, g4, Wh, bh, Wf, bf):
    """Full inputs -> list of per-core in_maps (numpy, contiguous)."""
    featg = _assemble_feat(cfg, g1, g2, g3, g4)
    bht = np.ascontiguousarray(
        np.asarray(bh, dtype=np.float32).reshape(cfg.nl, cfg.hid))

    Wh = np.asarray(Wh, dtype=np.float32)
    Wfr = np.asarray(Wf, dtype=np.float32).reshape(
        cfg.nl * cfg.ntile_per_head, cfg.tile_n2, cfg.hid)

    in_maps = []
    for c in range(cfg.n_cores):
        whc = Wh[c * cfg.rows:(c + 1) * cfg.rows, :]
        whp = np.zeros((cfg.rows, cfg.nk * 128), dtype=np.float32)
        whp[:, :cfg.feat_len] = whc
        wh_tiles = whp.reshape(cfg.rows, 128, cfg.nk).transpose(2, 1, 0)
        nAk = cfg.nA * cfg.g1k
        wh_a = np.ascontiguousarray(
            wh_tiles[:nAk].reshape(cfg.nA, cfg.g1k, 128, cfg.rows)
            .transpose(0, 2, 1, 3)
            .reshape(cfg.nA, 128, cfg.g1k * cfg.rows)).astype(BF16)
        wh_b = np.ascontiguousarray(
            wh_tiles[nAk:].reshape(cfg.nB, cfg.gB, 128, cfg.rows)
            .transpose(0, 2, 1, 3)
            .reshape(cfg.nB, 128, cfg.gB * cfg.rows)).astype(BF16)

        t0 = c * cfg.t2
        wf_tiles = Wfr[t0:t0 + cfg.t2].transpose(0, 2, 1)  # [t2, hid, n2]
        wf_g = np.ascontiguousarray(
            wf_tiles.reshape(cfg.ng2, cfg.g2k, cfg.hid, cfg.tile_n2)
            .transpose(0, 2, 1, 3)
            .reshape(cfg.ng2, cfg.hid, cfg.g2k * cfg.tile_n2)).astype(BF16)

        selm = np.zeros((cfg.nl, cfg.nruns), dtype=np.float32)
        for j in range(cfg.nruns):
            h = (t0 + j * cfg.run) // cfg.ntile_per_head
            selm[h, j] = 1.0

        in_maps.append({
            "featg": featg, "wha": wh_a, "whb": wh_b, "bht": bht,
            "sel": selm.astype(BF16), "wf": wf_g,
        })
    return in_maps


def unshard_output(cfg: Cfg, outs, bf):
    """outs: list of per-core [128, outc] -> [nl, 1, d2] (+ bf)."""
    # column c = u*nq + q holds output values [q*128 .. q*128+128) of
    # per-core tile u; row p is the value index within the chunk.
    parts = []
    for arr in outs:
        a = np.asarray(arr, dtype=np.float32).T  # [outc, 128]
        parts.append(a.reshape(cfg.t2, cfg.nq * 128))  # [126, 512]
    glob = np.concatenate(parts, axis=0)  # [1008, 512]
    res = glob.reshape(cfg.nl, cfg.d2)
    res = res + np.asarray(bf, dtype=np.float32).reshape(cfg.nl, cfg.d2)
    return np.ascontiguousarray(res[:, None, :])


_NC_CACHE = {}


def _get_nc(cfg: Cfg):
    if cfg not in _NC_CACHE:
        _NC_CACHE[cfg] = build_nc(cfg)
    return _NC_CACHE[cfg]


def kernel(g1, g2, g3, g4, Wh, bh, Wf, bf):
    from concourse import bass_utils

    cfg = FULL
    nc = _get_nc(cfg)
    in_maps = shard_inputs(cfg, g1, g2, g3, g4, Wh, bh, Wf, bf)
    res = bass_utils.run_bass_kernel_spmd(
        nc, in_maps, core_ids=list(range(cfg.n_cores)))
    return unshard_output(cfg, [res.results[c]["out"]
                                for c in range(cfg.n_cores)], bf)
